# revision 21
# baseline (speedup 1.0000x reference)
"""TRN2 Bass kernel v6 for nn_Attention_21758304322201 (sparse_attention).

Reference computation (B=32, L=2048, D=32, C=20):
    v = vals @ W_v.T
    k = LN(keys @ W_k.T);  q = LN(ques @ W_q.T)
    a = q @ k.T / sqrt(C);  a[masked keys] = -inf
    p = softmax(a);  o = p @ v
    out = LN(o + ques)

v5 (on top of v4's fp8 p/vals + one-matmul phase-1 variance):
  * phase-1 LN tail is a per-512-chunk pipeline: var(+eps) -> DVE
    reciprocal_approx_fast -> ACT Sqrt (bf16 rstd) -> DVE fold, emitted
    right after each side's chunk step so it hides under the remaining
    projection steps.  The serial Ln/Exp chain (13 us PE gap, 4 ACT
    table loads) is gone; phase 1 touches only the Sqrt table.
  * group_fin rstd = Sqrt(recip(var) * g_o^2): recip on DVE, one ACT
    Sqrt (g_o^2 folded into the activation scale) -- no more Ln+Exp
    table pair colliding with phase-2 ACT exps; zo/d1sq run on the
    otherwise-idle Pool engine.
  * o free-dim layout [n_be n_bo | o_be o_bo]: the normalizer replicas
    land on partitions 0:64, aligned with quesT (pair A) / an on-device
    quesLow copy (pair B), so the q*n residual term is 2 DVE ops and the
    1 MB pre-arranged quesT2 disappears.
  * exp engine pattern is per-qt tunable; qt4 (where group_fin(0) lands
    on ACT) runs a DVE-heavy pattern.
"""
import math

import numpy as np
import ml_dtypes

from concourse import bacc, bass, bass_utils, tile
from concourse import mybir

dt = mybir.dt
F32 = dt.float32
BF16 = dt.bfloat16
F8 = dt.float8e4
I8 = dt.int8
U8 = dt.uint8
U16 = dt.uint16
AO = mybir.AluOpType
AF = mybir.ActivationFunctionType

# problem constants (hardcoded per harness contract)
B, LQ, LK, D, C = 32, 2048, 2048, 32, 20
EPS = 1e-5
NCORES = 8
BPC = B // NCORES          # batches per core = 4
CAUG = C + 1               # 21-dim augmented projection (mean fold)
NT = 256                   # q-tile width
NQT = LQ // NT             # 8 q tiles
S20 = math.sqrt(C)
GS = 1.0 / S20             # score scale

# e4m3 Schraudolph (fp8 bit pattern): bits = round(x * A8 + B8) as int8
A8 = 8.0 / math.log(2.0)
B8 = 56.49
ONE8 = 0x38                # 1.0 in e4m3 bits

# exp engine per (qt, j): 'A' = ACT, 'D' = DVE. qt4 is DVE-heavy: the
# group_fin(0) ACT work (Sqrt + table) lands during its phase 2.
EXP_PATTERNS = (
    "ADADADADA", "DADADADAD", "ADADADADA", "DADADADAD",
    "DADADADAD", "ADADADADA", "DADADADAD", "ADADADADA",
)

PHASES = 3
_cache: dict = {}


def build_module(KC: int, reps: int = 1):
    """Build the SPMD bass module for per-core work. KC = padded key count."""
    NJ = KC // 128
    nc = bacc.Bacc("TRN2", target_bir_lowering=False, debug=False,
                   num_devices=NCORES)

    def din(name, shape, dty=F32):
        return nc.dram_tensor(name, shape, dty, kind="ExternalInput").ap()

    quesT_d = din("quesT", [128, LQ], U16)
    keysT_d = din("keysT", [128, KC], U16)
    valsP8_d = din("valsP8", [128, NJ * 512], U8)
    wq_d = din("wq_st", [128, 32])
    wk_d = din("wk_st", [128, 32])
    wvz_d = din("wvz", [128, 256])
    indvar_d = din("indvar", [128, 128])
    indz_mu_d = din("indz_mu", [128, 128])
    indz_sq_d = din("indz_sq", [128, 128])
    glb_d = din("glb", [128, 4])     # cols: g_o, b_o, g_o^2, (pad)
    qrow1_d = nc.dram_tensor("qrow1", [4, LQ], U16, kind="ExternalInput").ap()
    krow1_d = nc.dram_tensor("krow1", [4, KC], U16, kind="ExternalInput").ap()
    out_d = nc.dram_tensor("out", [128, LQ], F32, kind="ExternalOutput").ap()

    with tile.TileContext(nc) as tc:
        with tc.tile_pool(name="inp", bufs=1) as inp, \
             tc.tile_pool(name="cst", bufs=1) as cst:
            # ---- load inputs. Sync-queue order = first-need order.
            wk_f = cst.tile([128, 32], F32)
            nc.sync.dma_start(wk_f[:], wk_d)
            wq_f = cst.tile([128, 32], F32)
            nc.sync.dma_start(wq_f[:], wq_d)
            keysT_bf = inp.tile([128, KC], BF16)
            nc.sync.dma_start(keysT_bf[:, 0:512].bitcast(U16),
                              keysT_d[:, 0:512])
            indvar_f = cst.tile([128, 128], F32)
            nc.sync.dma_start(indvar_f[:], indvar_d)
            quesT_bf = inp.tile([128, LQ], BF16)
            nc.sync.dma_start(quesT_bf[:, 0:512].bitcast(U16),
                              quesT_d[:, 0:512])
            nc.sync.dma_start(keysT_bf[:, 512:KC].bitcast(U16),
                              keysT_d[:, 512:KC])
            nc.sync.dma_start(quesT_bf[:, 512:LQ].bitcast(U16),
                              quesT_d[:, 512:LQ])
            wvz_f = cst.tile([128, 256], F32)
            nc.sync.dma_start(wvz_f[:], wvz_d)
            indz_mu = cst.tile([128, 128], F32)
            nc.sync.dma_start(indz_mu[:], indz_mu_d)
            indz_sq_f = cst.tile([128, 128], F32)
            nc.sync.dma_start(indz_sq_f[:], indz_sq_d)
            glb = cst.tile([128, 4], F32)
            nc.sync.dma_start(glb[:], glb_d)
            # bulk late-use tensors ride the Pool DGE queue; quesLow is the
            # b2/b3 half of quesT shifted to partitions 0:64 (phase 3)
            valsP8_t = inp.tile([128, NJ * 512], U8)
            nc.gpsimd.dma_start(valsP8_t[:], valsP8_d)
            quesLow = inp.tile([64, LQ], BF16)
            nc.gpsimd.dma_start(quesLow[:].bitcast(U16),
                                quesT_bf[64:128, :].bitcast(U16))

            # ---- one-time constant prep ----
            wq_bf = cst.tile([128, 32], BF16)
            nc.vector.tensor_copy(wq_bf[:], wq_f[:])
            wk_bf = cst.tile([128, 32], BF16)
            nc.vector.tensor_copy(wk_bf[:], wk_f[:])
            wvz_bf = cst.tile([128, 256], BF16)
            nc.vector.tensor_copy(wvz_bf[:], wvz_f[:])
            indvar_bf = cst.tile([128, 128], BF16)
            nc.vector.tensor_copy(indvar_bf[:], indvar_f[:])
            indz_sq_bf = cst.tile([128, 128], BF16)
            nc.vector.tensor_copy(indz_sq_bf[:], indz_sq_f[:])
            eps_t = cst.tile([128, 1], F32)
            nc.gpsimd.memset(eps_t[:], EPS)
            # paired-layout k stationary; zero blocks + guard rows persist
            # across reps (per-rep folds only write rows rb+0:21)
            ksc2 = cst.tile([128, 2 * KC], BF16)
            nc.gpsimd.memset(ksc2[:], 0.0)
            for b in range(4):
                rb = 32 * (b % 2) + 64 * (b // 2)
                m = b % 2
                gdst = ksc2[rb + 21:rb + 22, :].rearrange(
                    "p (c k) -> p c k", k=128)[:, :, 64 * m:64 * m + 64]
                nc.sync.dma_start(
                    gdst.bitcast(U16),
                    krow1_d[b:b + 1, :].rearrange("p (c k) -> p c k", k=64))

            pk = dict(
                NJ=NJ, quesT_bf=quesT_bf, quesLow=quesLow,
                keysT_bf=keysT_bf, valsP8_t=valsP8_t, ksc2=ksc2,
                wq_bf=wq_bf, wk_bf=wk_bf, wvz_bf=wvz_bf,
                indvar_bf=indvar_bf, indz_mu=indz_mu,
                indz_sq_bf=indz_sq_bf, glb=glb, eps_t=eps_t, out_d=out_d,
                qrow1_d=qrow1_d, krow1_d=krow1_d,
            )

            if reps == 1:
                _body(nc, tc, pk)
            elif reps == 0:
                pass
            else:
                with tc.For_i(0, reps, 1):
                    _body(nc, tc, pk)

    nc.compile()
    return nc


class _Ph1Side:
    """One side (q or k) of phase 1, chunk-steppable for k/q interleaving.

    Per chunk: proj -> hat(bf16) -> sq -> var matmul -> var(+eps) stage ->
    DVE recip -> ACT sqrt (bf16 rstd). The fold into the score layout is
    side-specific and done by the caller.
    """

    def __init__(self, nc, pk, wk, sb1, pools, src_bf, w_bf, L, tg):
        self.nc, self.pk, self.L, self.tg = nc, pk, L, tg
        self.src_bf, self.w_bf = src_bf, w_bf
        self.hat_bf = wk.tile([128, L], BF16, tag=f"hat{tg}")
        self.sq_bf = sb1.tile([128, L], BF16, tag=f"sq{tg}")
        self.var_sb = sb1.tile([128, L], F32, tag=f"var{tg}")
        self.rstd_f = sb1.tile([128, L], F32, tag=f"rstd{tg}")
        self.sb1 = sb1
        self.wk = wk
        self.pools = pools       # shared (projp, varp)
        self.chunks = list(range(0, L, 512))

    def step(self, t0):
        nc = self.nc
        w = min(512, self.L - t0)
        projp, varp = self.pools
        proj_ps = projp.tile([128, 512], F32, tag="pp")
        for b in range(4):
            nc.tensor.matmul(
                proj_ps[32 * b:32 * b + 32, :w],
                self.w_bf[32 * b:32 * b + 32, :],
                self.src_bf[32 * b:32 * b + 32, t0:t0 + w],
                start=True, stop=True,
                tile_position=(32 * b, 32 * b),
            )
        nc.scalar.copy(self.hat_bf[:, t0:t0 + w], proj_ps[:, :w])
        nc.vector.tensor_tensor(self.sq_bf[:, t0:t0 + w],
                                self.hat_bf[:, t0:t0 + w],
                                self.hat_bf[:, t0:t0 + w], AO.mult)
        # var = E[x^2] - mu^2 in ONE matmul: the squared aug row is 20*mu^2
        var_ps = varp.tile([128, 512], F32, tag="var")
        nc.tensor.matmul(var_ps[:, :w], self.pk["indvar_bf"][:],
                         self.sq_bf[:, t0:t0 + w], start=True, stop=True)
        # rstd = recip(sqrt(var + eps)): ACT reads PSUM directly with the
        # eps floor in the bias (padded keys and the unused rows 22:31 have
        # var == 0; recip(0) would inject NaN through 0*NaN in the scores)
        nc.scalar.activation(self.var_sb[:, t0:t0 + w], var_ps[:, :w],
                             AF.Sqrt, bias=self.pk["eps_t"][:])
        nc.vector.reciprocal_approx_fast(self.rstd_f[:, t0:t0 + w],
                                         self.var_sb[:, t0:t0 + w])


def _body(nc, tc, pk):
    """One full forward pass for this core's 4 batches."""
    NJ = pk["NJ"]
    KC = NJ * 128
    quesLow, valsP8_t, ksc2 = pk["quesLow"], pk["valsP8_t"], pk["ksc2"]
    quesT_bf = pk["quesT_bf"]
    wvz_bf = pk["wvz_bf"]
    indz_mu, indz_sq_bf = pk["indz_mu"], pk["indz_sq_bf"]
    glb, out_d = pk["glb"], pk["out_d"]

    with tc.tile_pool(name="work", bufs=1) as wk:
        # ================= phase 1: projections + LN folds =================
        with tc.tile_pool(name="ph1sb", bufs=1) as sb1:
            with tc.tile_pool(name="pps", bufs=3, space="PSUM") as projp, \
                 tc.tile_pool(name="varps", bufs=2, space="PSUM") as varp:
                pools = (projp, varp)
                kside = _Ph1Side(nc, pk, wk, sb1, pools, pk["keysT_bf"],
                                 pk["wk_bf"], KC, "k")
                qside = _Ph1Side(nc, pk, wk, sb1, pools, pk["quesT_bf"],
                                 pk["wq_bf"], LQ, "q")
                qsc_bf = wk.tile([128, LQ], BF16, tag="scq")

                def k_fold(t0):
                    # ksc2 chunk-group for hat cols t0:t0+512
                    c0, c1 = t0 // 64, min(t0 + 512, KC) // 64
                    cs = slice(c0, c1)
                    for b in range(4):
                        rb = 32 * (b % 2) + 64 * (b // 2)
                        m = b % 2
                        dst = ksc2[rb:rb + 21, :].rearrange(
                            "p (c k) -> p c k",
                            k=128)[:, cs, 64 * m:64 * m + 64]
                        nc.vector.tensor_tensor(
                            dst,
                            kside.hat_bf[32 * b:32 * b + 21,
                                         t0:t0 + (c1 - c0) * 64].rearrange(
                                "p (c k) -> p c k", k=64),
                            kside.rstd_f[32 * b:32 * b + 21,
                                         t0:t0 + (c1 - c0) * 64].rearrange(
                                "p (c k) -> p c k", k=64),
                            AO.mult)

                def q_fold(t0):
                    eng = nc.vector if t0 == 0 else nc.gpsimd
                    eng.tensor_tensor(qsc_bf[:, t0:t0 + 512],
                                      qside.hat_bf[:, t0:t0 + 512],
                                      qside.rstd_f[:, t0:t0 + 512],
                                      AO.mult)
                    for b in range(4):
                        nc.sync.dma_start(
                            qsc_bf[32 * b + C + 1:32 * b + C + 2,
                                   t0:t0 + 512].bitcast(U16),
                            pk["qrow1_d"][b:b + 1, t0:t0 + 512])

                # interleave k/q chunk steps; each side's fold trails its
                # own step so everything pipelines across PE/ACT/DVE
                steps = []
                for i in range(max(len(kside.chunks), len(qside.chunks))):
                    if i < len(kside.chunks):
                        steps.append((kside, k_fold, kside.chunks[i]))
                    if i < len(qside.chunks):
                        steps.append((qside, q_fold, qside.chunks[i]))
                for side, fold, t0 in steps:
                    side.step(t0)
                    fold(t0)

        if PHASES < 2:
            return

        # ============ phase 2+3: attention + per-tile output LN ============
        with tc.tile_pool(name="scps", bufs=3, space="PSUM") as scps, \
             tc.tile_pool(name="oacc", bufs=1, space="PSUM") as oaccp, \
             tc.tile_pool(name="p3ps", bufs=1, space="PSUM") as p3p, \
             tc.tile_pool(name="psb", bufs=10) as psb, \
             tc.tile_pool(name="obfp", bufs=6) as obfp, \
             tc.tile_pool(name="ep", bufs=4) as ep, \
             tc.tile_pool(name="gp", bufs=1) as gp:

            o_banks_all = []   # per qt: o_pair (pair A cols 0:NT, B NT:2NT)

            def phase2(qt, mid_cb=None):
                t0 = qt * NT
                o_pair = oaccp.tile([128, 2 * NT], F32, tag="o")
                o_banks_all.append(o_pair)
                def emit_o(j, p_t):
                    # o accumulation: block-diagonal ones/vals stationaries
                    # (fp8), mixed-batch key partitions, all at (0,0)
                    for pair, h in ((0, 0), (1, 0), (0, 1), (1, 1)):
                        c0 = 512 * pair + 256 * h
                        blk = 512 * j + 256 * pair + 128 * h
                        p_sl = p_t[:, c0:c0 + 256].bitcast(F8)
                        nc.tensor.matmul(
                            o_pair[:, NT * pair:NT * pair + NT],
                            valsP8_t[:, blk:blk + 128].bitcast(F8),
                            p_sl,
                            start=(j == 0 and h == 0),
                            stop=(j == NJ - 1 and h == 1),
                            tile_position=(0, 0))

                pend = None   # o matmuls lag one j: the in-order PE queue
                # must see scores(j+1) before o(j), which waits on exp(j)
                for j in range(NJ):
                    scb = scps.tile([128, 4 * NT], F32, tag="scb")
                    # emission order A1,B1,A2,B2: pair A (rows 0:64) -> bank0
                    # (cols 0:512), pair B (rows 64:128) -> bank1; same-row
                    # matmuls serialize, cross-pair ones hit distinct banks.
                    for pair, h in ((0, 0), (1, 0), (0, 1), (1, 1)):
                        r0 = 64 * pair
                        c0 = 512 * pair + 256 * h
                        nc.tensor.matmul(
                            scb[:, c0:c0 + 256],
                            ksc2[r0:r0 + 64,
                                 128 * (2 * j + h):128 * (2 * j + h) + 128],
                            qsc_bf[r0:r0 + 64, t0:t0 + NT],
                            start=True, stop=True,
                            tile_position=(r0, 0),
                        )
                    # single wide exp over the whole j tile, fp8 out
                    eng = EXP_PATTERNS[qt % len(EXP_PATTERNS)][j % 9]
                    if eng == "A":
                        p_t = psb.tile([128, 4 * NT], F8, tag="p")
                        nc.scalar.activation(p_t[:], scb[:], AF.Exp,
                                             bias=0.0, scale=float(GS))
                    else:
                        p_t = psb.tile([128, 4 * NT], I8, tag="p")
                        nc.vector.tensor_scalar(
                            p_t[:], scb[:],
                            float(GS * A8), float(B8), AO.mult, AO.add)
                    if pend is not None:
                        emit_o(*pend)
                    pend = (j, p_t)
                    if j == 3 and mid_cb is not None:
                        # previous qt's LN matmuls slot here: their DVE/ACT
                        # deps are satisfied, so the in-order PE queue never
                        # stalls on them at the qt boundary
                        mid_cb()
                emit_o(*pend)

            GRP = 4            # q tiles per finalize batch
            d1s = {}           # qt -> d1 tile (alive until its group finalizes)
            var_grp = gp.tile([128, GRP * NT], F32, tag="vgrp")
            var_grp2 = gp.tile([128, GRP * NT], F32, tag="vgrp2")
            var_grps = [var_grp, var_grp2]

            heads = {}     # qt -> (o_bf0, o_bf1, t_sb)

            def phase3_head(qt):
                """The o_pair PSUM readers -- emitted before the next
                phase2 so the single o accumulator can recycle early."""
                t0 = qt * NT
                o_pair = o_banks_all[qt]
                # o -> SBUF bf16 for the z1 matmul (PE can't read PSUM);
                # layout per pair block: [n_be n_bo | o_be o_bo]
                o_bf0 = obfp.tile([128, NT], BF16, tag="obf")
                nc.scalar.copy(o_bf0[:], o_pair[:, 0:NT])
                o_bf1 = obfp.tile([128, NT], BF16, tag="obf")
                nc.scalar.copy(o_bf1[:], o_pair[:, NT:2 * NT])
                # t = ques * n: n replicas sit on partitions 0:64, aligned
                # with quesT (pair A) / quesLow (pair B); DVE writes the
                # pair-B result to out partitions 64:128 (partition shift)
                t_sb = ep.tile([128, NT], F32, tag="t")
                nc.vector.tensor_tensor(t_sb[0:64, :],
                                        quesT_bf[0:64, t0:t0 + NT],
                                        o_bf0[0:64, :], AO.mult)
                nc.vector.tensor_tensor(t_sb[64:128, :],
                                        quesLow[:, t0:t0 + NT],
                                        o_bf1[0:64, :], AO.mult)
                heads[qt] = (o_bf0, o_bf1, t_sb)

            def phase3_tail(qt):
                o_bf0, o_bf1, t_sb = heads.pop(qt)
                # z1 = W_v @ o_raw: two accumulating block-diagonal matmuls
                z1_ps = p3p.tile([128, NT], F32, tag="p3")
                nc.tensor.matmul(z1_ps[:], wvz_bf[:, 0:128], o_bf0[:],
                                 start=True, stop=False, tile_position=(0, 0))
                nc.tensor.matmul(z1_ps[:], wvz_bf[:, 128:256], o_bf1[:],
                                 start=False, stop=True, tile_position=(0, 0))
                z_f = ep.tile([128, NT], F32, tag="z")
                nc.vector.tensor_tensor(z_f[:], z1_ps[:], t_sb[:], AO.add)
                # output LN, centered moments, broadcast via stationaries
                mu_ps = p3p.tile([128, NT], F32, tag="p3")
                nc.tensor.matmul(mu_ps[:], indz_mu[:], z_f[:],
                                 start=True, stop=True)
                d1 = ep.tile([128, NT], F32, tag=f"d1_{qt % (GRP + 1)}")
                nc.vector.tensor_tensor(d1[:], z_f[:], mu_ps[:], AO.subtract)
                d1s[qt] = d1
                d1sq = ep.tile([128, NT], BF16, tag="d1sq")
                nc.vector.tensor_tensor(d1sq[:], d1[:], d1[:], AO.mult)
                var_ps = p3p.tile([128, NT], F32, tag="p3")
                nc.tensor.matmul(var_ps[:], indz_sq_bf[:], d1sq[:],
                                 start=True, stop=True)
                # stage var to SBUF (Copy needs no table load) for the group
                g0 = (qt // GRP) * GRP
                nc.scalar.copy(
                    var_grps[(qt // GRP) % 2][:, (qt - g0) * NT:
                                              (qt - g0 + 1) * NT],
                    var_ps[:])

            def group_fin(g0):
                """rstd = Sqrt(recip(var) * g_o^2) + zo, tiles g0..g0+GRP-1."""
                vg = var_grps[(g0 // GRP) % 2]
                rcp = gp.tile([128, GRP * NT], F32,
                              tag=f"rcp{(g0 // GRP) % 2}")
                nc.vector.reciprocal_approx_fast(rcp[:], vg[:])
                rstdg = gp.tile([128, GRP * NT], F32,
                                tag=f"rstd{(g0 // GRP) % 2}")
                nc.scalar.activation(rstdg[:], rcp[:], AF.Sqrt,
                                     scale=glb[:, 2:3])
                for qt in range(g0, g0 + GRP):
                    c0 = (qt - g0) * NT
                    zo = ep.tile([128, NT], F32, tag="zo")
                    nc.vector.tensor_tensor(zo[:], d1s[qt][:],
                                            rstdg[:, c0:c0 + NT], AO.mult)
                    nc.sync.dma_start(out_d[:, qt * NT:qt * NT + NT], zo[:])

            for qt in range(NQT):
                if PHASES >= 3 and qt > 0:
                    phase3_head(qt - 1)
                    phase2(qt, mid_cb=lambda q=qt - 1: phase3_tail(q))
                    if qt % GRP == 0:
                        group_fin(qt - GRP)
                else:
                    phase2(qt)
            if PHASES >= 3:
                phase3_head(NQT - 1)
                phase3_tail(NQT - 1)
                group_fin(NQT - GRP)


# ---------------------------------------------------------------------------
# host side
# ---------------------------------------------------------------------------

def _bf16_bits(x):
    u = np.ascontiguousarray(x, np.float32).view(np.uint32)
    return ((u + 0x7FFF + ((u >> 16) & 1)) >> 16).astype(np.uint16)


def _fp8_bits(x):
    return np.asarray(x, np.float32).astype(
        ml_dtypes.float8_e4m3).view(np.uint8)


def prepare_inputs(vals, keys, ques, key_mask, W_v, W_k, W_q,
                   g_k, b_k, g_q, b_q, g_o, b_o):
    """Shard + lay out the full inputs for the 8 cores. Returns (in_maps, KC)."""
    vals = np.ascontiguousarray(vals, np.float32)
    keys = np.ascontiguousarray(keys, np.float32)
    ques = np.ascontiguousarray(ques, np.float32)
    key_mask = np.asarray(key_mask)
    W_v = np.asarray(W_v, np.float32)
    W_k = np.asarray(W_k, np.float32)
    W_q = np.asarray(W_q, np.float32)
    g_k = np.asarray(g_k, np.float32)
    b_k = np.asarray(b_k, np.float32)
    g_q = np.asarray(g_q, np.float32)
    b_q = np.asarray(b_q, np.float32)
    g_o = np.asarray(g_o, np.float32)
    b_o = np.asarray(b_o, np.float32)

    # supported parameterization (holds for the harness inputs)
    if not (np.allclose(b_k, 0) and np.allclose(b_q, 0)):
        raise NotImplementedError("nonzero k/q LN bias not supported")
    if not (np.allclose(g_k, g_k.flat[0]) and np.allclose(g_q, g_q.flat[0])):
        raise NotImplementedError("non-uniform k/q LN gain not supported")
    if not (np.allclose(b_o, 0) and np.all(g_o > 0)):
        raise NotImplementedError("output LN with b_o!=0 or g_o<=0")
    guni = float(g_k.flat[0] * g_q.flat[0])

    counts = (~key_mask).sum(axis=1)
    KC = int(np.ceil(max(int(counts.max()), 1) / 128) * 128)
    NJ = KC // 128

    wq_aug = np.zeros((32, 32), np.float32)
    wq_aug[:, :C] = W_q.T
    wq_aug[:, C] = W_q.sum(axis=0) / S20
    wq_aug *= guni     # fold uniform LN gains into the q side
    wk_aug = np.zeros((32, 32), np.float32)
    wk_aug[:, :C] = W_k.T
    wk_aug[:, C] = -W_k.sum(axis=0) / S20

    wq_st = np.zeros((128, 32), np.float32)
    wk_st = np.zeros((128, 32), np.float32)
    wvz = np.zeros((128, 256), np.float32)
    # z1 stationaries: o sits on partitions 64:96 (b_even) / 96:128 (b_odd)
    # of each o_bf; z rows are the standard 32b batch layout
    wvz[64:96, 0:32] = W_v.T
    wvz[96:128, 32:64] = W_v.T
    wvz[64:96, 128 + 64:128 + 96] = W_v.T
    wvz[96:128, 128 + 96:128 + 128] = W_v.T
    indvar = np.zeros((128, 128), np.float32)
    indz_mu = np.zeros((128, 128), np.float32)
    indz_sq = np.zeros((128, 128), np.float32)
    glb = np.zeros((128, 4), np.float32)
    for b in range(BPC):
        r = 32 * b
        wq_st[r:r + 32] = wq_aug
        wk_st[r:r + 32] = wk_aug
        for c in range(CAUG + 1):
            # var = E[x^2] - mu^2: +1/C over the 20 dims, -1/C on the aug
            # row (whose square is 20*mu^2)
            indvar[r:r + C, r + c] = 1.0 / C
            indvar[r + C, r + c] = -1.0 / C
        indz_mu[r:r + 32, r:r + 32] = 1.0 / D
        indz_sq[r:r + 32, r:r + 32] = 1.0 / D
        glb[r:r + 32, 0] = g_o
        glb[r:r + 32, 1] = b_o
        glb[r:r + 32, 2] = g_o * g_o
    in_maps = []
    for cid in range(NCORES):
        quesT = np.zeros((128, LQ), np.uint16)
        keysT = np.zeros((128, KC), np.uint16)
        valsP8 = np.zeros((128, NJ * 512), np.uint8)
        krow1 = np.zeros((4, KC), np.uint16)
        for b in range(BPC):
            g = cid * BPC + b
            idx = np.flatnonzero(~key_mask[g])
            ci = len(idx)
            quesT[32 * b:32 * b + 32] = _bf16_bits(ques[g].T)
            keysT[32 * b:32 * b + 32, :ci] = _bf16_bits(keys[g][idx].T)
            vc = np.zeros((KC, D), np.float32)
            vc[:ci] = vals[g][idx]
            vcb = _fp8_bits(vc)
            pair, half = b // 2, b % 2
            for j in range(NJ):
                for h in range(2):
                    blk = 512 * j + 256 * pair + 128 * h
                    rows = slice(64 * half, 64 * half + 64)
                    # free layout [n_be n_bo | o_be o_bo]: ones at
                    # 32*half, vals at 64 + 32*half
                    valsP8[rows, blk + 32 * half:blk + 32 * half + 32] = ONE8
                    valsP8[rows, blk + 64 + 32 * half:
                           blk + 64 + 32 * half + 32] = \
                        vcb[128 * j + 64 * h:128 * j + 64 * h + 64]
            krow1[b, ci:] = np.float32(-300.0).view(np.uint32) >> 16
        in_maps.append({
            "quesT": quesT, "keysT": keysT, "valsP8": valsP8,
            "wq_st": wq_st, "wk_st": wk_st, "wvz": wvz,
            "indvar": indvar, "indz_mu": indz_mu, "indz_sq": indz_sq,
            "glb": glb,
            "qrow1": np.full((4, LQ), 0x3F80, np.uint16),
            "krow1": krow1,
        })
    return in_maps, KC


def unshard_output(results):
    out = np.empty((B, LQ, D), np.float32)
    for cid in range(NCORES):
        o = results[cid]["out"]
        for b in range(BPC):
            out[cid * BPC + b] = o[32 * b:32 * b + 32, :].T
    return out


def kernel(**inputs) -> np.ndarray:
    in_maps, KC = prepare_inputs(**inputs)
    key = ("nc", KC)
    if key not in _cache:
        _cache[key] = build_module(KC)
    nc = _cache[key]
    res = bass_utils.run_bass_kernel_spmd(nc, in_maps,
                                          core_ids=list(range(NCORES)))
    return unshard_output(res.results)


# revision 22
# speedup vs baseline: 1.2292x; 1.2292x over previous
"""TRN2 Bass kernel v6 for nn_Attention_21758304322201 (sparse_attention).

Reference computation (B=32, L=2048, D=32, C=20):
    v = vals @ W_v.T
    k = LN(keys @ W_k.T);  q = LN(ques @ W_q.T)
    a = q @ k.T / sqrt(C);  a[masked keys] = -inf
    p = softmax(a);  o = p @ v
    out = LN(o + ques)

v5 (on top of v4's fp8 p/vals + one-matmul phase-1 variance):
  * phase-1 LN tail is a per-512-chunk pipeline: var(+eps) -> DVE
    reciprocal_approx_fast -> ACT Sqrt (bf16 rstd) -> DVE fold, emitted
    right after each side's chunk step so it hides under the remaining
    projection steps.  The serial Ln/Exp chain (13 us PE gap, 4 ACT
    table loads) is gone; phase 1 touches only the Sqrt table.
  * group_fin rstd = Sqrt(recip(var) * g_o^2): recip on DVE, one ACT
    Sqrt (g_o^2 folded into the activation scale) -- no more Ln+Exp
    table pair colliding with phase-2 ACT exps; zo/d1sq run on the
    otherwise-idle Pool engine.
  * o free-dim layout [n_be n_bo | o_be o_bo]: the normalizer replicas
    land on partitions 0:64, aligned with quesT (pair A) / an on-device
    quesLow copy (pair B), so the q*n residual term is 2 DVE ops and the
    1 MB pre-arranged quesT2 disappears.
  * exp engine pattern is per-qt tunable; qt4 (where group_fin(0) lands
    on ACT) runs a DVE-heavy pattern.
"""
import math

import numpy as np
import ml_dtypes

from concourse import bacc, bass, bass_utils, tile
from concourse import mybir

dt = mybir.dt
F32 = dt.float32
BF16 = dt.bfloat16
F8 = dt.float8e4
I8 = dt.int8
U8 = dt.uint8
U16 = dt.uint16
AO = mybir.AluOpType
AF = mybir.ActivationFunctionType

# problem constants (hardcoded per harness contract)
B, LQ, LK, D, C = 32, 2048, 2048, 32, 20
EPS = 1e-5
NCORES = 8
BPC = B // NCORES          # batches per core = 4
CAUG = C + 1               # 21-dim augmented projection (mean fold)
NT = 256                   # q-tile width
NQT = LQ // NT             # 8 q tiles
S20 = math.sqrt(C)
GS = 1.0 / S20             # score scale

# e4m3 Schraudolph (fp8 bit pattern): bits = round(x * A8 + B8) as int8
A8 = 8.0 / math.log(2.0)
B8 = 56.49
ONE8 = 0x38                # 1.0 in e4m3 bits

# exp engine per (qt, j): 'A' = ACT, 'D' = DVE. qt4 is DVE-heavy: the
# group_fin(0) ACT work (Sqrt + table) lands during its phase 2.
EXP_PATTERNS = (
    "ADADADADA", "DADADADAD", "ADADADADA", "DADADADAD",
    "DADADADAD", "ADADADADA", "DADADADAD", "ADADADADA",
)

PHASES = 3
_cache: dict = {}


def build_module(KC: int, reps: int = 1):
    """Build the SPMD bass module for per-core work. KC = padded key count."""
    NJ = KC // 128
    nc = bacc.Bacc("TRN2", target_bir_lowering=False, debug=False,
                   num_devices=NCORES)

    def din(name, shape, dty=F32):
        return nc.dram_tensor(name, shape, dty, kind="ExternalInput").ap()

    quesT_d = din("quesT", [128, LQ], U16)
    keysT_d = din("keysT", [128, KC], U16)
    valsP8_d = din("valsP8", [128, NJ * 512], U8)
    wq_d = din("wq_st", [128, 32])
    wk_d = din("wk_st", [128, 32])
    wvz_d = din("wvz", [128, 256])
    indvar_d = din("indvar", [128, 128])
    indz_mu_d = din("indz_mu", [128, 128])
    indz_sq_d = din("indz_sq", [128, 128])
    glb_d = din("glb", [128, 4])     # cols: g_o, b_o, g_o^2, (pad)
    qrow1_d = nc.dram_tensor("qrow1", [4, LQ], U16, kind="ExternalInput").ap()
    krow1_d = nc.dram_tensor("krow1", [4, KC], U16, kind="ExternalInput").ap()
    out_d = nc.dram_tensor("out", [128, LQ], F32, kind="ExternalOutput").ap()

    with tile.TileContext(nc) as tc:
        with tc.tile_pool(name="inp", bufs=1) as inp, \
             tc.tile_pool(name="cst", bufs=1) as cst:
            # ---- load inputs. Sync-queue order = first-need order.
            wk_f = cst.tile([128, 32], F32)
            nc.sync.dma_start(wk_f[:], wk_d)
            wq_f = cst.tile([128, 32], F32)
            nc.sync.dma_start(wq_f[:], wq_d)
            keysT_bf = inp.tile([128, KC], BF16)
            nc.sync.dma_start(keysT_bf[:, 0:512].bitcast(U16),
                              keysT_d[:, 0:512])
            indvar_f = cst.tile([128, 128], F32)
            nc.sync.dma_start(indvar_f[:], indvar_d)
            quesT_bf = inp.tile([128, LQ], BF16)
            nc.sync.dma_start(quesT_bf[:, 0:512].bitcast(U16),
                              quesT_d[:, 0:512])
            nc.sync.dma_start(keysT_bf[:, 512:KC].bitcast(U16),
                              keysT_d[:, 512:KC])
            nc.sync.dma_start(quesT_bf[:, 512:LQ].bitcast(U16),
                              quesT_d[:, 512:LQ])
            wvz_f = cst.tile([128, 256], F32)
            nc.sync.dma_start(wvz_f[:], wvz_d)
            indz_mu = cst.tile([128, 128], F32)
            nc.sync.dma_start(indz_mu[:], indz_mu_d)
            indz_sq_f = cst.tile([128, 128], F32)
            nc.sync.dma_start(indz_sq_f[:], indz_sq_d)
            glb = cst.tile([128, 4], F32)
            nc.sync.dma_start(glb[:], glb_d)
            # bulk late-use tensors ride the Pool DGE queue; quesLow is the
            # b2/b3 half of quesT shifted to partitions 0:64 (phase 3)
            valsP8_t = inp.tile([128, NJ * 512], U8)
            nc.gpsimd.dma_start(valsP8_t[:], valsP8_d)
            quesLow = inp.tile([64, LQ], BF16)
            nc.gpsimd.dma_start(quesLow[:].bitcast(U16),
                                quesT_bf[64:128, :].bitcast(U16))

            # ---- one-time constant prep ----
            wq_bf = cst.tile([128, 32], BF16)
            nc.vector.tensor_copy(wq_bf[:], wq_f[:])
            wk_bf = cst.tile([128, 32], BF16)
            nc.vector.tensor_copy(wk_bf[:], wk_f[:])
            wvz_bf = cst.tile([128, 256], BF16)
            nc.vector.tensor_copy(wvz_bf[:], wvz_f[:])
            indvar_bf = cst.tile([128, 128], BF16)
            nc.vector.tensor_copy(indvar_bf[:], indvar_f[:])
            indz_sq_bf = cst.tile([128, 128], BF16)
            nc.vector.tensor_copy(indz_sq_bf[:], indz_sq_f[:])
            eps_t = cst.tile([128, 1], F32)
            nc.gpsimd.memset(eps_t[:], EPS)
            # paired-layout k stationary; zero blocks + guard rows persist
            # across reps (per-rep folds only write rows rb+0:21)
            ksc2 = cst.tile([128, 2 * KC], BF16)
            nc.gpsimd.memset(ksc2[:], 0.0)
            for b in range(4):
                rb = 32 * (b % 2) + 64 * (b // 2)
                m = b % 2
                gdst = ksc2[rb + 21:rb + 22, :].rearrange(
                    "p (c k) -> p c k", k=128)[:, :, 64 * m:64 * m + 64]
                nc.sync.dma_start(
                    gdst.bitcast(U16),
                    krow1_d[b:b + 1, :].rearrange("p (c k) -> p c k", k=64))

            pk = dict(
                NJ=NJ, quesT_bf=quesT_bf, quesLow=quesLow,
                keysT_bf=keysT_bf, valsP8_t=valsP8_t, ksc2=ksc2,
                wq_bf=wq_bf, wk_bf=wk_bf, wvz_bf=wvz_bf,
                indvar_bf=indvar_bf, indz_mu=indz_mu,
                indz_sq_bf=indz_sq_bf, glb=glb, eps_t=eps_t, out_d=out_d,
                qrow1_d=qrow1_d, krow1_d=krow1_d,
            )

            if reps == 1:
                _body(nc, tc, pk)
            elif reps == 0:
                pass
            else:
                with tc.For_i(0, reps, 1):
                    _body(nc, tc, pk)

    nc.compile()
    return nc


class _Ph1Side:
    """One side (q or k) of phase 1, chunk-steppable for k/q interleaving.

    Per chunk: proj -> hat(bf16) -> sq -> var matmul -> var(+eps) stage ->
    DVE recip -> ACT sqrt (bf16 rstd). The fold into the score layout is
    side-specific and done by the caller.
    """

    def __init__(self, nc, pk, wk, sb1, pools, src_bf, w_bf, L, tg):
        self.nc, self.pk, self.L, self.tg = nc, pk, L, tg
        self.src_bf, self.w_bf = src_bf, w_bf
        self.hat_bf = wk.tile([128, L], BF16, tag=f"hat{tg}")
        self.sq_bf = sb1.tile([128, L], BF16, tag=f"sq{tg}")
        self.var_sb = sb1.tile([128, L], F32, tag=f"var{tg}")
        self.rstd_f = sb1.tile([128, L], F32, tag=f"rstd{tg}")
        self.sb1 = sb1
        self.wk = wk
        self.pools = pools       # shared (projp, varp)
        self.chunks = list(range(0, L, 512))

    def step(self, t0):
        nc = self.nc
        w = min(512, self.L - t0)
        projp, varp = self.pools
        proj_ps = projp.tile([128, 512], F32, tag="pp")
        for b in range(4):
            nc.tensor.matmul(
                proj_ps[32 * b:32 * b + 32, :w],
                self.w_bf[32 * b:32 * b + 32, :],
                self.src_bf[32 * b:32 * b + 32, t0:t0 + w],
                start=True, stop=True,
                tile_position=(32 * b, 32 * b),
            )
        nc.scalar.copy(self.hat_bf[:, t0:t0 + w], proj_ps[:, :w])
        nc.vector.tensor_tensor(self.sq_bf[:, t0:t0 + w],
                                self.hat_bf[:, t0:t0 + w],
                                self.hat_bf[:, t0:t0 + w], AO.mult)
        # var = E[x^2] - mu^2 in ONE matmul: the squared aug row is 20*mu^2
        var_ps = varp.tile([128, 512], F32, tag="var")
        nc.tensor.matmul(var_ps[:, :w], self.pk["indvar_bf"][:],
                         self.sq_bf[:, t0:t0 + w], start=True, stop=True)
        # rstd = recip(sqrt(var + eps)): ACT reads PSUM directly with the
        # eps floor in the bias (padded keys and the unused rows 22:31 have
        # var == 0; recip(0) would inject NaN through 0*NaN in the scores)
        nc.scalar.activation(self.var_sb[:, t0:t0 + w], var_ps[:, :w],
                             AF.Sqrt, bias=self.pk["eps_t"][:])
        nc.vector.reciprocal_approx_fast(self.rstd_f[:, t0:t0 + w],
                                         self.var_sb[:, t0:t0 + w])


def _body(nc, tc, pk):
    """One full forward pass for this core's 4 batches."""
    NJ = pk["NJ"]
    KC = NJ * 128
    quesLow, valsP8_t, ksc2 = pk["quesLow"], pk["valsP8_t"], pk["ksc2"]
    quesT_bf = pk["quesT_bf"]
    wvz_bf = pk["wvz_bf"]
    indz_mu, indz_sq_bf = pk["indz_mu"], pk["indz_sq_bf"]
    glb, out_d = pk["glb"], pk["out_d"]

    with tc.tile_pool(name="work", bufs=1) as wk:
        # ================= phase 1: projections + LN folds =================
        with tc.tile_pool(name="ph1sb", bufs=1) as sb1:
            with tc.tile_pool(name="pps", bufs=3, space="PSUM") as projp, \
                 tc.tile_pool(name="varps", bufs=2, space="PSUM") as varp:
                pools = (projp, varp)
                kside = _Ph1Side(nc, pk, wk, sb1, pools, pk["keysT_bf"],
                                 pk["wk_bf"], KC, "k")
                qside = _Ph1Side(nc, pk, wk, sb1, pools, pk["quesT_bf"],
                                 pk["wq_bf"], LQ, "q")
                qsc_bf = wk.tile([128, LQ], BF16, tag="scq")

                def k_fold(t0):
                    # ksc2 chunk-group for hat cols t0:t0+512
                    c0, c1 = t0 // 64, min(t0 + 512, KC) // 64
                    cs = slice(c0, c1)
                    for b in range(4):
                        rb = 32 * (b % 2) + 64 * (b // 2)
                        m = b % 2
                        dst = ksc2[rb:rb + 21, :].rearrange(
                            "p (c k) -> p c k",
                            k=128)[:, cs, 64 * m:64 * m + 64]
                        nc.vector.tensor_tensor(
                            dst,
                            kside.hat_bf[32 * b:32 * b + 21,
                                         t0:t0 + (c1 - c0) * 64].rearrange(
                                "p (c k) -> p c k", k=64),
                            kside.rstd_f[32 * b:32 * b + 21,
                                         t0:t0 + (c1 - c0) * 64].rearrange(
                                "p (c k) -> p c k", k=64),
                            AO.mult)

                def q_fold(t0):
                    eng = nc.vector if t0 == 0 else nc.gpsimd
                    eng.tensor_tensor(qsc_bf[:, t0:t0 + 512],
                                      qside.hat_bf[:, t0:t0 + 512],
                                      qside.rstd_f[:, t0:t0 + 512],
                                      AO.mult)
                    for b in range(4):
                        nc.sync.dma_start(
                            qsc_bf[32 * b + C + 1:32 * b + C + 2,
                                   t0:t0 + 512].bitcast(U16),
                            pk["qrow1_d"][b:b + 1, t0:t0 + 512])

                # interleave k/q chunk steps; each side's fold trails its
                # own step so everything pipelines across PE/ACT/DVE
                steps = []
                for i in range(max(len(kside.chunks), len(qside.chunks))):
                    if i < len(kside.chunks):
                        steps.append((kside, k_fold, kside.chunks[i]))
                    if i < len(qside.chunks):
                        steps.append((qside, q_fold, qside.chunks[i]))
                for side, fold, t0 in steps:
                    side.step(t0)
                    fold(t0)

        if PHASES < 2:
            return

        # ============ phase 2+3: attention + per-tile output LN ============
        with tc.tile_pool(name="scps", bufs=3, space="PSUM") as scps, \
             tc.tile_pool(name="oacc", bufs=1, space="PSUM") as oaccp, \
             tc.tile_pool(name="p3ps", bufs=1, space="PSUM") as p3p, \
             tc.tile_pool(name="psb", bufs=10) as psb, \
             tc.tile_pool(name="obfp", bufs=6) as obfp, \
             tc.tile_pool(name="ep", bufs=4) as ep, \
             tc.tile_pool(name="gp", bufs=1) as gp:

            o_banks_all = []   # per qt: o_pair (pair A cols 0:NT, B NT:2NT)

            def phase2(qt):
                t0 = qt * NT
                o_pair = oaccp.tile([128, 2 * NT], F32, tag="o")
                o_banks_all.append(o_pair)
                def emit_o(j, p_t):
                    # o accumulation: block-diagonal ones/vals stationaries
                    # (fp8), mixed-batch key partitions, all at (0,0)
                    for pair, h in ((0, 0), (1, 0), (0, 1), (1, 1)):
                        c0 = 512 * pair + 256 * h
                        blk = 512 * j + 256 * pair + 128 * h
                        p_sl = p_t[:, c0:c0 + 256].bitcast(F8)
                        nc.tensor.matmul(
                            o_pair[:, NT * pair:NT * pair + NT],
                            valsP8_t[:, blk:blk + 128].bitcast(F8),
                            p_sl,
                            start=(j == 0 and h == 0),
                            stop=(j == NJ - 1 and h == 1),
                            tile_position=(0, 0))

                pend = None   # o matmuls lag one j: the in-order PE queue
                # must see scores(j+1) before o(j), which waits on exp(j)
                for j in range(NJ):
                    scb = scps.tile([128, 4 * NT], F32, tag="scb")
                    # emission order A1,B1,A2,B2: pair A (rows 0:64) -> bank0
                    # (cols 0:512), pair B (rows 64:128) -> bank1; same-row
                    # matmuls serialize, cross-pair ones hit distinct banks.
                    for pair, h in ((0, 0), (1, 0), (0, 1), (1, 1)):
                        r0 = 64 * pair
                        c0 = 512 * pair + 256 * h
                        nc.tensor.matmul(
                            scb[:, c0:c0 + 256],
                            ksc2[r0:r0 + 64,
                                 128 * (2 * j + h):128 * (2 * j + h) + 128],
                            qsc_bf[r0:r0 + 64, t0:t0 + NT],
                            start=True, stop=True,
                            tile_position=(r0, 0),
                        )
                    # single wide exp over the whole j tile, fp8 out
                    eng = EXP_PATTERNS[qt % len(EXP_PATTERNS)][j % 9]
                    if eng == "A":
                        p_t = psb.tile([128, 4 * NT], F8, tag="p")
                        nc.scalar.activation(p_t[:], scb[:], AF.Exp,
                                             bias=0.0, scale=float(GS))
                    else:
                        p_t = psb.tile([128, 4 * NT], I8, tag="p")
                        nc.vector.tensor_scalar(
                            p_t[:], scb[:],
                            float(GS * A8), float(B8), AO.mult, AO.add)
                    if pend is not None:
                        emit_o(*pend)
                    pend = (j, p_t)
                emit_o(*pend)

            GRP = 4            # q tiles per finalize batch
            d1s = {}           # qt -> d1 tile (alive until its group finalizes)
            var_grp = gp.tile([128, GRP * NT], F32, tag="vgrp")
            var_grp2 = gp.tile([128, GRP * NT], F32, tag="vgrp2")
            var_grps = [var_grp, var_grp2]

            heads = {}     # qt -> (o_bf0, o_bf1, t_sb)

            def phase3_head(qt):
                """The o_pair PSUM readers -- emitted before the next
                phase2 so the single o accumulator can recycle early."""
                t0 = qt * NT
                o_pair = o_banks_all[qt]
                # o -> SBUF bf16 for the z1 matmul (PE can't read PSUM);
                # layout per pair block: [n_be n_bo | o_be o_bo]
                o_bf0 = obfp.tile([128, NT], BF16, tag="obf")
                nc.scalar.copy(o_bf0[:], o_pair[:, 0:NT])
                o_bf1 = obfp.tile([128, NT], BF16, tag="obf")
                nc.scalar.copy(o_bf1[:], o_pair[:, NT:2 * NT])
                # t = ques * n: n replicas sit on partitions 0:64, aligned
                # with quesT (pair A) / quesLow (pair B); DVE writes the
                # pair-B result to out partitions 64:128 (partition shift)
                t_sb = ep.tile([128, NT], F32, tag="t")
                nc.vector.tensor_tensor(t_sb[0:64, :],
                                        quesT_bf[0:64, t0:t0 + NT],
                                        o_bf0[0:64, :], AO.mult)
                nc.vector.tensor_tensor(t_sb[64:128, :],
                                        quesLow[:, t0:t0 + NT],
                                        o_bf1[0:64, :], AO.mult)
                heads[qt] = (o_bf0, o_bf1, t_sb)

            def phase3_tail(qt):
                o_bf0, o_bf1, t_sb = heads.pop(qt)
                # z1 = W_v @ o_raw: two accumulating block-diagonal matmuls
                z1_ps = p3p.tile([128, NT], F32, tag="p3")
                nc.tensor.matmul(z1_ps[:], wvz_bf[:, 0:128], o_bf0[:],
                                 start=True, stop=False, tile_position=(0, 0))
                nc.tensor.matmul(z1_ps[:], wvz_bf[:, 128:256], o_bf1[:],
                                 start=False, stop=True, tile_position=(0, 0))
                z_f = ep.tile([128, NT], F32, tag="z")
                nc.vector.tensor_tensor(z_f[:], z1_ps[:], t_sb[:], AO.add)
                # output LN, centered moments, broadcast via stationaries
                mu_ps = p3p.tile([128, NT], F32, tag="p3")
                nc.tensor.matmul(mu_ps[:], indz_mu[:], z_f[:],
                                 start=True, stop=True)
                d1 = ep.tile([128, NT], F32, tag=f"d1_{qt % (GRP + 1)}")
                nc.vector.tensor_tensor(d1[:], z_f[:], mu_ps[:], AO.subtract)
                d1s[qt] = d1
                d1sq = ep.tile([128, NT], BF16, tag="d1sq")
                nc.vector.tensor_tensor(d1sq[:], d1[:], d1[:], AO.mult)
                var_ps = p3p.tile([128, NT], F32, tag="p3")
                nc.tensor.matmul(var_ps[:], indz_sq_bf[:], d1sq[:],
                                 start=True, stop=True)
                # stage var to SBUF (Copy needs no table load) for the group
                g0 = (qt // GRP) * GRP
                nc.scalar.copy(
                    var_grps[(qt // GRP) % 2][:, (qt - g0) * NT:
                                              (qt - g0 + 1) * NT],
                    var_ps[:])

            def group_fin(g0):
                """rstd = Sqrt(recip(var) * g_o^2) + zo, tiles g0..g0+GRP-1."""
                vg = var_grps[(g0 // GRP) % 2]
                rcp = gp.tile([128, GRP * NT], F32,
                              tag=f"rcp{(g0 // GRP) % 2}")
                nc.vector.reciprocal_approx_fast(rcp[:], vg[:])
                rstdg = gp.tile([128, GRP * NT], F32,
                                tag=f"rstd{(g0 // GRP) % 2}")
                nc.scalar.activation(rstdg[:], rcp[:], AF.Sqrt,
                                     scale=glb[:, 2:3])
                for qt in range(g0, g0 + GRP):
                    c0 = (qt - g0) * NT
                    zo = ep.tile([128, NT], F32, tag="zo")
                    nc.vector.tensor_tensor(zo[:], d1s[qt][:],
                                            rstdg[:, c0:c0 + NT], AO.mult)
                    nc.sync.dma_start(out_d[:, qt * NT:qt * NT + NT], zo[:])

            for qt in range(NQT):
                if PHASES >= 3 and qt > 0:
                    phase3_head(qt - 1)
                phase2(qt)
                if PHASES >= 3 and qt > 0:
                    phase3_tail(qt - 1)
                    if qt % GRP == 0:
                        group_fin(qt - GRP)
            if PHASES >= 3:
                phase3_head(NQT - 1)
                phase3_tail(NQT - 1)
                group_fin(NQT - GRP)


# ---------------------------------------------------------------------------
# host side
# ---------------------------------------------------------------------------

def _bf16_bits(x):
    u = np.ascontiguousarray(x, np.float32).view(np.uint32)
    return ((u + 0x7FFF + ((u >> 16) & 1)) >> 16).astype(np.uint16)


def _fp8_bits(x):
    return np.asarray(x, np.float32).astype(
        ml_dtypes.float8_e4m3).view(np.uint8)


def prepare_inputs(vals, keys, ques, key_mask, W_v, W_k, W_q,
                   g_k, b_k, g_q, b_q, g_o, b_o):
    """Shard + lay out the full inputs for the 8 cores. Returns (in_maps, KC)."""
    vals = np.ascontiguousarray(vals, np.float32)
    keys = np.ascontiguousarray(keys, np.float32)
    ques = np.ascontiguousarray(ques, np.float32)
    key_mask = np.asarray(key_mask)
    W_v = np.asarray(W_v, np.float32)
    W_k = np.asarray(W_k, np.float32)
    W_q = np.asarray(W_q, np.float32)
    g_k = np.asarray(g_k, np.float32)
    b_k = np.asarray(b_k, np.float32)
    g_q = np.asarray(g_q, np.float32)
    b_q = np.asarray(b_q, np.float32)
    g_o = np.asarray(g_o, np.float32)
    b_o = np.asarray(b_o, np.float32)

    # supported parameterization (holds for the harness inputs)
    if not (np.allclose(b_k, 0) and np.allclose(b_q, 0)):
        raise NotImplementedError("nonzero k/q LN bias not supported")
    if not (np.allclose(g_k, g_k.flat[0]) and np.allclose(g_q, g_q.flat[0])):
        raise NotImplementedError("non-uniform k/q LN gain not supported")
    if not (np.allclose(b_o, 0) and np.all(g_o > 0)):
        raise NotImplementedError("output LN with b_o!=0 or g_o<=0")
    guni = float(g_k.flat[0] * g_q.flat[0])

    counts = (~key_mask).sum(axis=1)
    KC = int(np.ceil(max(int(counts.max()), 1) / 128) * 128)
    NJ = KC // 128

    wq_aug = np.zeros((32, 32), np.float32)
    wq_aug[:, :C] = W_q.T
    wq_aug[:, C] = W_q.sum(axis=0) / S20
    wq_aug *= guni     # fold uniform LN gains into the q side
    wk_aug = np.zeros((32, 32), np.float32)
    wk_aug[:, :C] = W_k.T
    wk_aug[:, C] = -W_k.sum(axis=0) / S20

    wq_st = np.zeros((128, 32), np.float32)
    wk_st = np.zeros((128, 32), np.float32)
    wvz = np.zeros((128, 256), np.float32)
    # z1 stationaries: o sits on partitions 64:96 (b_even) / 96:128 (b_odd)
    # of each o_bf; z rows are the standard 32b batch layout
    wvz[64:96, 0:32] = W_v.T
    wvz[96:128, 32:64] = W_v.T
    wvz[64:96, 128 + 64:128 + 96] = W_v.T
    wvz[96:128, 128 + 96:128 + 128] = W_v.T
    indvar = np.zeros((128, 128), np.float32)
    indz_mu = np.zeros((128, 128), np.float32)
    indz_sq = np.zeros((128, 128), np.float32)
    glb = np.zeros((128, 4), np.float32)
    for b in range(BPC):
        r = 32 * b
        wq_st[r:r + 32] = wq_aug
        wk_st[r:r + 32] = wk_aug
        for c in range(CAUG + 1):
            # var = E[x^2] - mu^2: +1/C over the 20 dims, -1/C on the aug
            # row (whose square is 20*mu^2)
            indvar[r:r + C, r + c] = 1.0 / C
            indvar[r + C, r + c] = -1.0 / C
        indz_mu[r:r + 32, r:r + 32] = 1.0 / D
        indz_sq[r:r + 32, r:r + 32] = 1.0 / D
        glb[r:r + 32, 0] = g_o
        glb[r:r + 32, 1] = b_o
        glb[r:r + 32, 2] = g_o * g_o
    in_maps = []
    for cid in range(NCORES):
        quesT = np.zeros((128, LQ), np.uint16)
        keysT = np.zeros((128, KC), np.uint16)
        valsP8 = np.zeros((128, NJ * 512), np.uint8)
        krow1 = np.zeros((4, KC), np.uint16)
        for b in range(BPC):
            g = cid * BPC + b
            idx = np.flatnonzero(~key_mask[g])
            ci = len(idx)
            quesT[32 * b:32 * b + 32] = _bf16_bits(ques[g].T)
            keysT[32 * b:32 * b + 32, :ci] = _bf16_bits(keys[g][idx].T)
            vc = np.zeros((KC, D), np.float32)
            vc[:ci] = vals[g][idx]
            vcb = _fp8_bits(vc)
            pair, half = b // 2, b % 2
            for j in range(NJ):
                for h in range(2):
                    blk = 512 * j + 256 * pair + 128 * h
                    rows = slice(64 * half, 64 * half + 64)
                    # free layout [n_be n_bo | o_be o_bo]: ones at
                    # 32*half, vals at 64 + 32*half
                    valsP8[rows, blk + 32 * half:blk + 32 * half + 32] = ONE8
                    valsP8[rows, blk + 64 + 32 * half:
                           blk + 64 + 32 * half + 32] = \
                        vcb[128 * j + 64 * h:128 * j + 64 * h + 64]
            krow1[b, ci:] = np.float32(-300.0).view(np.uint32) >> 16
        in_maps.append({
            "quesT": quesT, "keysT": keysT, "valsP8": valsP8,
            "wq_st": wq_st, "wk_st": wk_st, "wvz": wvz,
            "indvar": indvar, "indz_mu": indz_mu, "indz_sq": indz_sq,
            "glb": glb,
            "qrow1": np.full((4, LQ), 0x3F80, np.uint16),
            "krow1": krow1,
        })
    return in_maps, KC


def unshard_output(results):
    out = np.empty((B, LQ, D), np.float32)
    for cid in range(NCORES):
        o = results[cid]["out"]
        for b in range(BPC):
            out[cid * BPC + b] = o[32 * b:32 * b + 32, :].T
    return out


def kernel(**inputs) -> np.ndarray:
    in_maps, KC = prepare_inputs(**inputs)
    key = ("nc", KC)
    if key not in _cache:
        _cache[key] = build_module(KC)
    nc = _cache[key]
    res = bass_utils.run_bass_kernel_spmd(nc, in_maps,
                                          core_ids=list(range(NCORES)))
    return unshard_output(res.results)
